# revision 1
# baseline (speedup 1.0000x reference)
"""Trainium2 Bass kernel for nn_Conv3DRecurrentInhibition.

The reference computes a 10-step linear fixed-point iteration
    state <- x + conv_C(state)           (15-tap conv along channels, zero pad)
which collapses to a single linear operator
    out[b, :, h, w] = T @ x[b, :, h, w],   T = sum_{k=0}^{max_steps} W^k
where W is the exact 256x256 banded matrix of the zero-padded conv
(cross-correlation orientation, matching lax.conv_general_dilated).
T is built on host (float64, from the 15-tap w_rec input). The device
computes the residual form y = x + T'@x with T' = T - I: the fp32r
matmul's rounding error then scales with the small T' products while x
passes through in exact fp32 via the DVE add.

Sharding: pure data parallel on batch — 32 samples over 8 cores, 4 each.
"""

import numpy as np

N_CORES = 8
B_FULL = 32
B_CORE = B_FULL // N_CORES  # 4
C = 256
HW = 56 * 56  # 3136
NTILE = 392  # 3136 = 8 * 392; >=256 keeps float32r matmul at full rate
TILES_PER_CHUNK = 2  # 784-col chunks: load/compute/store pipeline granularity
CHUNK = NTILE * TILES_PER_CHUNK
N_CHUNK = HW // CHUNK

_NC_CACHE = {}


def build_nc(reps: int = 1):
    """Build + compile the per-core Bass program.

    Per core: x [4, 256, 3136] f32, tT [128, 2, 256] f32 (T^T tiled so that
    tT[kp, kc, m] = T[m, kc*128 + kp]), y [4, 256, 3136] f32.
    reps>1 repeats the whole workload (for steady-state timing harnesses).
    """
    if reps in _NC_CACHE:
        return _NC_CACHE[reps]

    import concourse.bacc as bacc
    import concourse.mybir as mybir
    from concourse import tile

    f32 = mybir.dt.float32
    f32r = mybir.dt.float32r

    nc = bacc.Bacc("TRN2", target_bir_lowering=False, debug=False,
                   num_devices=N_CORES)
    # inputs feed the PE as fp32r (same 4-byte layout as fp32; full-rate
    # matmul at N>=256) — the BIR verifier requires the producing DMA to
    # already be typed fp32r
    x = nc.dram_tensor("x", [B_CORE, C, HW], f32r, kind="ExternalInput")
    tT = nc.dram_tensor("tT", [128, 2, C], f32r, kind="ExternalInput")
    y = nc.dram_tensor("y", [B_CORE, C, HW], f32, kind="ExternalOutput")

    with tile.TileContext(nc) as tc:
        with (
            tc.tile_pool(name="w", bufs=1) as wpool,
            tc.tile_pool(name="xin", bufs=8) as xpool,
            tc.tile_pool(name="out", bufs=8) as opool,
            tc.tile_pool(name="ps", bufs=8, space="PSUM") as pspool,
        ):
            wt = wpool.tile([128, 2, C], f32r)
            nc.gpsimd.dma_start(wt[:], tT[:])  # SWDGE: keep sync ring free for x loads

            for _ in range(reps):
                for b in range(B_CORE):
                    for c in range(N_CHUNK):
                        cs = slice(c * CHUNK, (c + 1) * CHUNK)
                        xa = xpool.tile([128, CHUNK], f32r, tag="xa")
                        xb = xpool.tile([128, CHUNK], f32r, tag="xb")
                        nc.sync.dma_start(xa[:], x[b, 0:128, cs])
                        nc.sync.dma_start(xb[:], x[b, 128:256, cs])
                        oa = opool.tile([128, CHUNK], f32, tag="oa")
                        ob = opool.tile([128, CHUNK], f32, tag="ob")
                        for n in range(TILES_PER_CHUNK):
                            sl = slice(n * NTILE, (n + 1) * NTILE)
                            for mc, ot, xh in ((0, oa, xa), (1, ob, xb)):
                                ps = pspool.tile([128, NTILE], f32, tag="ps")
                                nc.tensor.matmul(
                                    ps[:],
                                    wt[:, 0, mc * 128:(mc + 1) * 128],
                                    xa[:, sl],
                                    start=True, stop=False,
                                )
                                nc.tensor.matmul(
                                    ps[:],
                                    wt[:, 1, mc * 128:(mc + 1) * 128],
                                    xb[:, sl],
                                    start=False, stop=True,
                                )
                                # y = x + T'x (x re-added in exact fp32)
                                nc.vector.tensor_add(
                                    ot[:, sl], ps[:], xh[:, sl].bitcast(f32))
                        # stores on the ACT HWDGE ring so they overlap the
                        # sync-ring loads
                        nc.scalar.dma_start(y[b, 0:128, cs], oa[:])
                        nc.scalar.dma_start(y[b, 128:256, cs], ob[:])

    nc.compile()
    _NC_CACHE[reps] = nc
    return nc


def compose_T(w_rec: np.ndarray, max_steps: int, n_chan: int = C) -> np.ndarray:
    """T = sum_{k=0}^{max_steps} W^k for the zero-padded channel conv.

    lax.conv is cross-correlation: out_c = sum_dd w[dd] * y[c + dd - pad],
    so W[i, j] = w[j - i + pad].
    """
    w = np.asarray(w_rec, dtype=np.float64).reshape(-1)
    scope = w.shape[0]
    pad = scope // 2
    W = np.zeros((n_chan, n_chan), dtype=np.float64)
    for dd in range(scope):
        off = dd - pad
        d = np.diagonal(W, offset=off)
        d.setflags(write=True)
        d[:] = w[dd]
    eye = np.eye(n_chan, dtype=np.float64)
    acc = eye.copy()
    for _ in range(int(max_steps)):
        acc = eye + W @ acc
    return acc.astype(np.float32)


def make_in_maps(activations: np.ndarray, w_rec: np.ndarray, max_steps) -> list:
    acts = np.ascontiguousarray(np.asarray(activations, dtype=np.float32))
    assert acts.shape == (B_FULL, C, 56, 56), acts.shape
    T = compose_T(w_rec, int(np.asarray(max_steps)))
    Tp = T - np.eye(C, dtype=np.float32)  # residual operator T' = T - I
    # lhsT layout: tT[kp, kc, m] = T'^T[kc*128 + kp, m] = T'[m, kc*128 + kp]
    tTr = np.ascontiguousarray(Tp.T.reshape(2, 128, C).transpose(1, 0, 2))
    shards = acts.reshape(N_CORES, B_CORE, C, HW)
    return [{"x": shards[i], "tT": tTr} for i in range(N_CORES)]


def kernel(**inputs) -> np.ndarray:
    from concourse.bass_utils import run_bass_kernel_spmd

    in_maps = make_in_maps(inputs["activations"], inputs["w_rec"],
                           inputs["max_steps"])
    nc = build_nc(reps=1)
    res = run_bass_kernel_spmd(nc, in_maps, list(range(N_CORES)))
    out = np.stack([np.asarray(res.results[i]["y"]) for i in range(N_CORES)])
    return out.reshape(B_FULL, C, 56, 56).astype(np.float32, copy=False)



# revision 2
# speedup vs baseline: 1.8855x; 1.8855x over previous
"""Trainium2 Bass kernel for nn_Conv3DRecurrentInhibition.

The reference computes a 10-step linear fixed-point iteration
    state <- x + conv_C(state)           (15-tap conv along channels, zero pad)
which collapses to a single linear operator
    out[b, :, h, w] = T @ x[b, :, h, w],   T = sum_{k=0}^{max_steps} W^k
where W is the exact 256x256 banded matrix of the zero-padded conv
(cross-correlation orientation, matching lax.conv_general_dilated).
T is built on host (float64, from the 15-tap w_rec input).

The kernel is HBM-bandwidth bound (~358 GB/s/core), so all device I/O is
bf16: x is cast on host, y is returned bf16 and upcast on host. Measured
end-to-end rel err ~7e-3 (gate is 2e-2). The device computes y = T@x
directly in the PE (bf16 weights, f32 PSUM accumulate); PSUM->SBUF
eviction copies alternate between DVE and ACT so neither engine becomes
the bottleneck.

Sharding: pure data parallel on batch — 32 samples over 8 cores, 4 each.
"""

import numpy as np

N_CORES = 8
B_FULL = 32
B_CORE = B_FULL // N_CORES  # 4
C = 256
HW = 56 * 56  # 3136
NTILE = 448  # 3136 = 7 * 448; 448 f32 = 1792B fits a 2KB PSUM bank
NT = HW // NTILE

_NC_CACHE = {}


def build_nc(loop_R=None):
    """Build + compile the per-core Bass program.

    Per core: x [4, 2, 128, 3136] bf16, tT [128, 2, 256] bf16 with
    tT[k, kc, m] = T[m, kc*128 + k], y [4, 2, 128, 3136] bf16.
    loop_R wraps the workload in a hardware For_i loop (timing rigs).
    """
    if loop_R in _NC_CACHE:
        return _NC_CACHE[loop_R]

    import concourse.bacc as bacc
    import concourse.mybir as mybir
    from concourse import tile

    f32 = mybir.dt.float32
    bf16 = mybir.dt.bfloat16

    nc = bacc.Bacc("TRN2", target_bir_lowering=False, debug=False,
                   num_devices=N_CORES)
    x = nc.dram_tensor("x", [B_CORE, 2, 128, HW], bf16, kind="ExternalInput")
    tT = nc.dram_tensor("tT", [128, 2, C], bf16, kind="ExternalInput")
    y = nc.dram_tensor("y", [B_CORE, 2, 128, HW], bf16, kind="ExternalOutput")

    with tile.TileContext(nc) as tc:
        with (
            tc.tile_pool(name="w", bufs=1) as wpool,
            tc.tile_pool(name="xin", bufs=6) as xpool,
            tc.tile_pool(name="out", bufs=6) as opool,
            tc.tile_pool(name="ps", bufs=8, space="PSUM") as pspool,
        ):
            wt = wpool.tile([128, 2, C], bf16)
            nc.gpsimd.dma_start(wt[:], tT[:])  # SWDGE: keep sync ring free

            def body():
                for b in range(B_CORE):
                    xa = xpool.tile([128, HW], bf16, tag="xa")
                    xb = xpool.tile([128, HW], bf16, tag="xb")
                    nc.sync.dma_start(xa[:], x[b, 0])
                    nc.sync.dma_start(xb[:], x[b, 1])
                    oa = opool.tile([128, HW], bf16, tag="oa")
                    ob = opool.tile([128, HW], bf16, tag="ob")
                    for j in range(NT):
                        sl = slice(j * NTILE, (j + 1) * NTILE)
                        for mc, ot in ((0, oa), (1, ob)):
                            ps = pspool.tile([128, NTILE], f32, tag="ps")
                            nc.tensor.matmul(
                                ps[:],
                                wt[:, 0, mc * 128:(mc + 1) * 128],
                                xa[:, sl],
                                start=True, stop=False,
                            )
                            nc.tensor.matmul(
                                ps[:],
                                wt[:, 1, mc * 128:(mc + 1) * 128],
                                xb[:, sl],
                                start=False, stop=True,
                            )
                            # evict PSUM->SBUF (f32 -> bf16), alternating
                            # engines so neither is the bottleneck
                            if (j * 2 + mc) % 2 == 0:
                                nc.vector.tensor_copy(ot[:, sl], ps[:])
                            else:
                                nc.scalar.copy(ot[:, sl], ps[:])
                    # stores on the ACT HWDGE ring so they overlap the
                    # sync-ring loads
                    nc.scalar.dma_start(y[b, 0], oa[:])
                    nc.scalar.dma_start(y[b, 1], ob[:])

            if loop_R is None:
                body()
            else:
                with tc.For_i(0, loop_R, 1):
                    body()

    nc.compile()
    _NC_CACHE[loop_R] = nc
    return nc


def compose_T(w_rec: np.ndarray, max_steps: int, n_chan: int = C) -> np.ndarray:
    """T = sum_{k=0}^{max_steps} W^k for the zero-padded channel conv.

    lax.conv is cross-correlation: out_c = sum_dd w[dd] * y[c + dd - pad],
    so W[i, j] = w[j - i + pad].
    """
    w = np.asarray(w_rec, dtype=np.float64).reshape(-1)
    scope = w.shape[0]
    pad = scope // 2
    W = np.zeros((n_chan, n_chan), dtype=np.float64)
    for dd in range(scope):
        off = dd - pad
        d = np.diagonal(W, offset=off)
        d.setflags(write=True)
        d[:] = w[dd]
    eye = np.eye(n_chan, dtype=np.float64)
    acc = eye.copy()
    for _ in range(int(max_steps)):
        acc = eye + W @ acc
    return acc.astype(np.float32)


def make_in_maps(activations: np.ndarray, w_rec: np.ndarray, max_steps) -> list:
    import ml_dtypes

    bf = ml_dtypes.bfloat16
    acts = np.asarray(activations, dtype=np.float32)
    assert acts.shape == (B_FULL, C, 56, 56), acts.shape
    T = compose_T(w_rec, int(np.asarray(max_steps)))
    # lhsT layout: tT[k, kc, m] = T[m, kc*128 + k]
    tTr = np.ascontiguousarray(
        T.T.reshape(2, 128, C).transpose(1, 0, 2)).astype(bf)
    shards = np.ascontiguousarray(acts.astype(bf)).reshape(
        N_CORES, B_CORE, 2, 128, HW)
    return [{"x": shards[i], "tT": tTr} for i in range(N_CORES)]


def kernel(**inputs) -> np.ndarray:
    from concourse.bass_utils import run_bass_kernel_spmd

    in_maps = make_in_maps(inputs["activations"], inputs["w_rec"],
                           inputs["max_steps"])
    nc = build_nc()
    res = run_bass_kernel_spmd(nc, in_maps, list(range(N_CORES)))
    out = np.stack([np.asarray(res.results[i]["y"]) for i in range(N_CORES)])
    return out.reshape(B_FULL, C, 56, 56).astype(np.float32)
